# revision 33
# baseline (speedup 1.0000x reference)
"""Bahdanau temporal attention on 8 Trainium2 NeuronCores.

Full-input contract: kernel(**inputs) takes the unsharded numpy arrays
(query (32,1024), keys (32,4096,1024), Wq (1024,512), Wk (1024,512),
v (512,)) and returns the full output (32,1,1024) float32.

Sharding: data-parallel over batch. Each of the 8 cores processes 4
batches; Wq/Wk/v are replicated. No collectives.

Per-core algorithm (B_loc=4, S=4096, H=1024, A=512):
  q_t    = query @ Wq                 (B_loc, A)
  k_t    = keys @ Wk                  (B_loc, S, A)
  energy = v . tanh(q_t + k_t)        (B_loc, S)
  w      = exp(energy)   (unnormalized; |energy| <= |v|_1 so exp cannot
                          overflow fp32 and no max-subtraction is needed)
  ctx    = (w @ keys) / sum(w)        (B_loc, H)

Single pass over keys, software-pipelined per 512-row S-tile with PE-queue
ordering [proj_i, energy_i, ctx_{i-1}] so the tensor engine never
head-of-line blocks on Act/DMA results:
  - keys tile loaded via SWDGE cast-DMA (f32 DRAM -> bf16 SBUF in flight,
    16KB descriptors), so no separate DVE cast pass and no f32 landing,
  - one xbar DMA-transpose pair (3D-out form) -> keysT with the H
    contraction dim on partitions,
  - PE: k_t^T = Wk^T @ keys^T (bf16, f32 PSUM accum),
  - ACT: T = tanh(k_t^T + q_t^T), q_t as per-partition bias,
  - PE: energy in NATURAL form: 4 accumulating matmuls with vT16 (16 cols,
    15 zero) stationary -> psum [16, S-tile]; rows 1..15 are zero,
  - ACT: w16 = exp(energy) into SBUF bf16; accum_out gives the per-tile
    softmax normalizer Z for free (f32),
  - sync queue: tiny xbar transpose w16 [16,512] -> wT [128, SC, 16],
  - PE: ctx += wT.T @ keys_bf (2 matmuls of N=512 per s-chunk).
All PE operands are bf16 (fp32 matmuls are 4x slower; fp8 DoubleRow fails
the 2e-2 relative-error gate: measured 2.5e-2 offline).
"""

import sys

if "/opt/trn_rl_repo" not in sys.path:
    sys.path.insert(0, "/opt/trn_rl_repo")

import numpy as np

import concourse.bass as bass
import concourse.tile as tile
from concourse import bacc
from concourse import mybir
from concourse.bass_utils import run_bass_kernel_spmd

F32 = mybir.dt.float32
BF16 = mybir.dt.bfloat16

N_CORES = 8
B, S, H, A = 32, 4096, 1024, 512
B_LOC = B // N_CORES          # 4 batches per core
ST = 512                      # S-tile rows
N_ST = S // ST                # 8 S-tiles per batch
P = 128                       # partitions
HC = H // P                   # 8 contraction chunks
AC = A // P                   # 4 a-chunks
SC = ST // P                  # 4 s-chunks per S-tile


def build_bass():
    nc = bacc.Bacc()

    d_query = nc.declare_dram_parameter("query", [B_LOC, H], F32, isOutput=False)
    d_keys = nc.declare_dram_parameter("keys", [B_LOC, S, H], F32, isOutput=False)
    d_wq = nc.declare_dram_parameter("Wq", [H, A], F32, isOutput=False)
    d_wk = nc.declare_dram_parameter("Wk", [H, A], F32, isOutput=False)
    d_v = nc.declare_dram_parameter("v", [A], F32, isOutput=False)
    d_out = nc.declare_dram_parameter("out", [B_LOC, H], F32, isOutput=True)

    from contextlib import ExitStack

    with tile.TileContext(nc) as tc, ExitStack() as ctx:
        build_kernel_body(tc, d_query, d_keys, d_wq, d_wk, d_v, d_out, ctx)
    nc.compile()
    return nc


def build_kernel_body(tc, d_query, d_keys, d_wq, d_wk, d_v, d_out, ctx):
    nc = tc.nc

    consts = ctx.enter_context(tc.tile_pool(name="consts", bufs=1))
    keybf = ctx.enter_context(tc.tile_pool(name="keybf", bufs=12))
    keytp = ctx.enter_context(tc.tile_pool(name="keytp", bufs=6))
    tp = ctx.enter_context(tc.tile_pool(name="tp", bufs=3))
    wp = ctx.enter_context(tc.tile_pool(name="wp", bufs=3))
    wtp = ctx.enter_context(tc.tile_pool(name="wtp", bufs=3))
    zp = ctx.enter_context(tc.tile_pool(name="zp", bufs=2))
    smalls = ctx.enter_context(tc.tile_pool(name="smalls", bufs=4))
    pp_kt = ctx.enter_context(tc.tile_pool(name="pp_kt", bufs=3, space="PSUM"))
    pp_e = ctx.enter_context(tc.tile_pool(name="pp_e", bufs=1, space="PSUM"))
    pp_ctx = ctx.enter_context(tc.tile_pool(name="pp_ctx", bufs=2, space="PSUM"))
    pp_wt = ctx.enter_context(tc.tile_pool(name="pp_wt", bufs=2, space="PSUM"))

    # ---- constants ----
    # All const xbar transposes go on the SCALAR queue so the sync queue is
    # reserved purely for the big keys transposes (no head-of-line blocking
    # of xpose(0) behind the q_t preamble chain at startup). Cross-engine
    # (DVE producer -> Act-queue DMA) ordering gets explicit semaphores.
    # Wk in bf16, laid out [h' (part), hc, a]; SWDGE casts f32 -> bf16 in flight
    wk_bf = consts.tile([P, HC, A], BF16)
    # Wq staged through the keybf pool (same 8KB/partition shape); only needed
    # for the one-time q_t preamble, and the pool rotation reclaims it.
    wq_sb = keybf.tile([P, HC, A], BF16, tag="kbf", name="wq_sb")
    v_f32 = consts.tile([1, A], F32)
    q_f32 = consts.tile([B_LOC, H], F32)

    def load_consts_front():
        nc.gpsimd.dma_start(out=wk_bf, in_=d_w_rearr(d_wk))

    def load_consts_rest():
        nc.gpsimd.dma_start(out=wq_sb, in_=d_w_rearr(d_wq))
        nc.gpsimd.dma_start(out=v_f32, in_=d_v[None, :])
        nc.gpsimd.dma_start(out=q_f32, in_=d_query[:, :])

    # v funnel: DVE-cast into row 0 of a 16-row tile, then xbar.
    # vT16 [128, AC, 16]: column 0 of each a-chunk is v, columns 1..15 are
    # zero -- used directly as the stationary operand of the energy matmul
    # (M=16 output partitions, rows 1..15 of the psum are zero).
    v16 = consts.tile([16, A], BF16)
    vT16 = consts.tile([P, AC, 16], BF16)
    q16 = consts.tile([16, H], BF16)
    qT16 = consts.tile([P, HC, 16], BF16)

    def build_const_funnels():
        nc.vector.memset(v16, 0.0)
        nc.vector.tensor_copy(v16[0:1, :], v_f32)
        nc.scalar.dma_start(out=vT16, in_=v16, transpose=True)
        nc.vector.memset(q16, 0.0)
        nc.vector.tensor_copy(q16[0:B_LOC, :], q_f32)
        nc.scalar.dma_start(out=qT16, in_=q16, transpose=True)

    # q_t = query @ Wq : psum (16, A), accumulate over hc
    qt16 = consts.tile([16, A], BF16)
    qtT16 = consts.tile([P, AC, 16], BF16)

    def build_qt():
        ps_qt = pp_e.tile([16, A], F32, tag="pe")
        for hc in range(HC):
            nc.tensor.matmul(
                ps_qt,
                lhsT=qT16[:, hc, :],
                rhs=wq_sb[:, hc, :],
                start=(hc == 0),
                stop=(hc == HC - 1),
            )
        nc.vector.memset(qt16, 0.0)
        nc.vector.tensor_copy(qt16[0:B_LOC, :], ps_qt[0:B_LOC, :])
        # xbar -> qtT16 (128, AC, 16); tanh bias per (ac, b) = qtT16[:, ac, b]
        nc.scalar.dma_start(out=qtT16, in_=qt16, transpose=True)

    # identity for the PE transpose of w16
    ident16 = consts.tile([16, 16], BF16)

    def build_ident():
        from concourse.masks import make_identity

        make_identity(nc, ident16)

    # ---- main loop: software-pipelined emission ----
    iters = [(b, st) for b in range(B_LOC) for st in range(N_ST)]
    n = len(iters)
    kbf_store = {}
    kT_store = {}
    w16_store = {}
    wT_store = {}
    ctx_psums = {}
    zaccs = {}

    def load(i):
        # SWDGE cast-DMA: f32 DRAM -> bf16 SBUF in flight (gpsimd queue)
        b, st = iters[i]
        kb = keybf.tile([P, SC, H], BF16, tag="kbf")
        nc.gpsimd.dma_start(
            out=kb,
            in_=d_keys[b, st * ST : (st + 1) * ST, :].rearrange(
                "(p r) h -> p r h", p=P
            ),
        )
        kbf_store[i] = kb

    def xpose(i):
        # both halves on the sync queue: the xbar is a single exclusive
        # hardware unit -- concurrent ucode transposes from two queues
        # interleave on it and corrupt the output (measured 1.7e-1 rel err).
        # The queue is only held for descriptor-gen (~0.6us); the transfer
        # itself is async, so one queue can pipeline transposes fine as long
        # as they are issued with enough lead.
        kb = kbf_store[i]
        kT = keytp.tile([P, SC, HC, P], BF16, tag="kT")
        for j in range(2):
            nc.sync.dma_start(
                out=kT[:, 2 * j : 2 * j + 2, :, :],
                in_=kb[:, 2 * j : 2 * j + 2, :],
                transpose=True,
            )
        kT_store[i] = kT

    def proj_energy(i):
        b, st = iters[i]
        kT = kT_store.pop(i)

        if st == 0:
            zaccs[b] = zp.tile([1, N_ST], F32, tag="z", name="zacc")

        # projection + tanh: T[a' (part), ac, s]
        T_sb = tp.tile([P, AC, ST], BF16, tag="T")
        for ac in range(AC):
            ps_kt = pp_kt.tile([P, ST], F32, tag="kt")
            for hc in range(HC):
                nc.tensor.matmul(
                    ps_kt,
                    lhsT=wk_bf[:, hc, ac * P : (ac + 1) * P],
                    rhs=kT[:, :, hc, :],
                    start=(hc == 0),
                    stop=(hc == HC - 1),
                )
            nc.scalar.activation(
                T_sb[:, ac, :],
                ps_kt,
                mybir.ActivationFunctionType.Tanh,
                bias=qtT16[:, ac, b : b + 1],
            )

        # energy natural: ps_e [16, ST], row 0 = v . T, rows 1..15 = 0
        ps_e = pp_e.tile([16, ST], F32, tag="pe")
        for ac in range(AC):
            nc.tensor.matmul(
                ps_e,
                lhsT=vT16[:, ac, :],
                rhs=T_sb[:, ac, :],
                start=(ac == 0),
                stop=(ac == AC - 1),
            )

        # w16 = exp(energy); row 0 is the real w, rows 1..15 are exp(0)=1
        w16 = wp.tile([16, ST], BF16, tag="w16")
        nc.scalar.activation(
            w16,
            ps_e,
            mybir.ActivationFunctionType.Exp,
        )
        # per-tile softmax-normalizer partial: Z_st = sum_s w16[0, s]
        nc.vector.tensor_reduce(
            out=zaccs[b][0:1, st : st + 1],
            in_=w16[0:1, :],
            axis=mybir.AxisListType.X,
            op=mybir.AluOpType.add,
        )
        w16_store[i] = w16

    def wt_make(i):
        # w16 [16, 512] -> wT [128, SC, 16] (column 0 carries w) via PE
        # transpose (4 tiny is_transpose matmuls into a bf16 psum) + one DVE
        # psum->sbuf copy. No DMA queue involved, and the cross-engine hops
        # (Act exp -> PE -> DVE -> PE ctx) all get real semaphores.
        w16 = w16_store.pop(i)
        ps_wt = pp_wt.tile([P, SC, 16], BF16, tag="pwt")
        for sc in range(SC):
            nc.tensor.matmul(
                ps_wt[:, sc, :],
                lhsT=w16[:, sc * P : (sc + 1) * P],
                rhs=ident16,
                is_transpose=True,
            )
        wT = wtp.tile([P, SC, 16], BF16, tag="wT")
        nc.vector.tensor_copy(wT, ps_wt)
        wT_store[i] = wT

    def ctx_acc(i):
        b, st = iters[i]
        first = st == 0
        last = st == N_ST - 1
        if first:
            ps_c0 = pp_ctx.tile([1, 512], F32, tag="ctx")
            ps_c1 = pp_ctx.tile([1, 512], F32, tag="ctx")
            ctx_psums[b] = (ps_c0, ps_c1)
        ps_c0, ps_c1 = ctx_psums[b]
        wT = wT_store.pop(i)
        kb = kbf_store.pop(i)
        for sc in range(SC):
            nc.tensor.matmul(
                ps_c0,
                lhsT=wT[:, sc, 0:1],
                rhs=kb[:, sc, 0:512],
                start=(first and sc == 0),
                stop=(last and sc == SC - 1),
            )
            nc.tensor.matmul(
                ps_c1,
                lhsT=wT[:, sc, 0:1],
                rhs=kb[:, sc, 512:1024],
                start=(first and sc == 0),
                stop=(last and sc == SC - 1),
            )
        if last:
            finalize(b, ps_c0, ps_c1)

    def finalize(b, ps_c0, ps_c1):
        # Z = sum over the 8 per-tile partials (row 0 of zacc), out = ctx / Z
        zrow = zaccs.pop(b)
        zsum = smalls.tile([1, 1], F32, tag="Z")
        nc.vector.tensor_reduce(
            out=zsum,
            in_=zrow[0:1, :],
            axis=mybir.AxisListType.X,
            op=mybir.AluOpType.add,
        )
        rz = smalls.tile([1, 1], F32, tag="rz")
        nc.vector.reciprocal(rz, zsum)
        out_sb = smalls.tile([1, H], F32, tag="out")
        nc.vector.tensor_scalar_mul(out_sb[0:1, 0:512], ps_c0, rz)
        nc.vector.tensor_scalar_mul(out_sb[0:1, 512:1024], ps_c1, rz)
        nc.gpsimd.dma_start(out=d_out[b : b + 1, :], in_=out_sb)

    # schedule: loads lead by 4, transposes by 2; PE slot order is
    # [proj_i, energy_i, ctx_{i-1}] so PE never waits on Act/DMA results.
    # Prologue interleaves keys loads with const loads on the gpsimd queue
    # so tile 0's chain starts immediately.
    load(0)
    load(1)
    load_consts_front()   # wk (needed by proj_0)
    load_consts_rest()    # wq, v, q
    load(2)
    load(3)
    load(4)
    load(5)
    build_ident()
    build_const_funnels()
    build_qt()
    xpose(0)
    xpose(1)
    xpose(2)
    for i in range(n + 1):
        if i < n:
            if i + 6 < n:
                load(i + 6)
            if i + 3 < n:
                xpose(i + 3)
            if i >= 1:
                wt_make(i - 1)
            proj_energy(i)
            if i >= 1:
                ctx_acc(i - 1)
        else:
            wt_make(n - 1)
            ctx_acc(n - 1)


def d_w_rearr(d_w):
    # (H, A) dram -> [h' (part), hc, a] view
    return d_w.rearrange("(hc p) a -> p hc a", p=P)


_CACHED_NC = None


def _get_nc():
    global _CACHED_NC
    if _CACHED_NC is None:
        _CACHED_NC = build_bass()
    return _CACHED_NC


def kernel(query, keys, Wq, Wk, v):
    query = np.ascontiguousarray(np.asarray(query, dtype=np.float32))
    keys = np.ascontiguousarray(np.asarray(keys, dtype=np.float32))
    Wq = np.ascontiguousarray(np.asarray(Wq, dtype=np.float32))
    Wk = np.ascontiguousarray(np.asarray(Wk, dtype=np.float32))
    v = np.ascontiguousarray(np.asarray(v, dtype=np.float32))

    nc = _get_nc()
    in_maps = []
    for c in range(N_CORES):
        sl = slice(c * B_LOC, (c + 1) * B_LOC)
        in_maps.append(
            {
                "query": query[sl],
                "keys": keys[sl],
                "Wq": Wq,
                "Wk": Wk,
                "v": v,
            }
        )
    last_err = None
    for attempt in range(3):
        try:
            res = run_bass_kernel_spmd(nc, in_maps, list(range(N_CORES)))
            out = np.concatenate(
                [np.asarray(res.results[c]["out"]) for c in range(N_CORES)], axis=0
            )
            break
        except Exception as e:  # transient device-unrecoverable states heal on retry
            last_err = e
            import time

            time.sleep(5)
    else:
        raise last_err
    return out.reshape(B, 1, H).astype(np.float32)


if __name__ == "__main__":
    rng = np.random.default_rng(0)
    q = rng.standard_normal((B, H), dtype=np.float32)
    k = rng.standard_normal((B, S, H), dtype=np.float32)
    wq = rng.standard_normal((H, A), dtype=np.float32) / np.sqrt(H)
    wk = rng.standard_normal((H, A), dtype=np.float32) / np.sqrt(H)
    vv = rng.standard_normal((A,), dtype=np.float32) / np.sqrt(A)
    o = kernel(query=q, keys=k, Wq=wq, Wk=wk, v=vv)
    print(o.shape, o.dtype)


# revision 34
# speedup vs baseline: 1.1453x; 1.1453x over previous
"""Bahdanau temporal attention on 8 Trainium2 NeuronCores.

Full-input contract: kernel(**inputs) takes the unsharded numpy arrays
(query (32,1024), keys (32,4096,1024), Wq (1024,512), Wk (1024,512),
v (512,)) and returns the full output (32,1,1024) float32.

Sharding: data-parallel over batch. Each of the 8 cores processes 4
batches; Wq/Wk/v are replicated. No collectives.

Per-core algorithm (B_loc=4, S=4096, H=1024, A=512):
  q_t    = query @ Wq                 (B_loc, A)
  k_t    = keys @ Wk                  (B_loc, S, A)
  energy = v . tanh(q_t + k_t)        (B_loc, S)
  w      = exp(energy)   (unnormalized; |energy| <= |v|_1 so exp cannot
                          overflow fp32 and no max-subtraction is needed)
  ctx    = (w @ keys) / sum(w)        (B_loc, H)

Single pass over keys, software-pipelined per 512-row S-tile:
  - DMA keys tile f32 (p-major layout -> 16KB descriptors), DVE-cast bf16,
  - one xbar DMA-transpose pair (3D-out form) -> keysT with the H
    contraction dim on partitions,
  - PE: k_t^T = Wk^T @ keys^T (bf16, f32 PSUM accum),
  - ACT: T = tanh(k_t^T + q_t^T), q_t as per-partition bias,
  - PE: energy^T via (K=a, M=s-chunk, N=1) matmuls (already s-on-partitions),
  - ACT: w^T = exp(energy^T) straight from PSUM into SBUF,
  - PE: ctx += w^T.T @ keys_bf and Z += w^T.T @ ones — numerator and
    normalizer use identical bf16 weights, so quantization largely cancels.
All PE operands are bf16 (fp32 matmuls are self-loading + 4x slower;
fp32r requires producer-side rounding walrus verifies).
"""

import sys

if "/opt/trn_rl_repo" not in sys.path:
    sys.path.insert(0, "/opt/trn_rl_repo")

import numpy as np

import concourse.bass as bass
import concourse.tile as tile
from concourse import bacc
from concourse import mybir
from concourse.bass_utils import run_bass_kernel_spmd
from concourse.masks import make_identity

F32 = mybir.dt.float32
F32R = mybir.dt.float32r
BF16 = mybir.dt.bfloat16

N_CORES = 8
B, S, H, A = 32, 4096, 1024, 512
B_LOC = B // N_CORES          # 4 batches per core
ST = 512                      # S-tile rows
N_ST = S // ST                # 8 S-tiles per batch
P = 128                       # partitions
HC = H // P                   # 8 contraction chunks
AC = A // P                   # 4 a-chunks
SC = ST // P                  # 4 s-chunks per S-tile


def build_bass():
    nc = bacc.Bacc()

    d_query = nc.declare_dram_parameter("query", [B_LOC, H], F32, isOutput=False)
    d_keys = nc.declare_dram_parameter("keys", [B_LOC, S, H], F32, isOutput=False)
    d_wq = nc.declare_dram_parameter("Wq", [H, A], F32, isOutput=False)
    d_wk = nc.declare_dram_parameter("Wk", [H, A], F32, isOutput=False)
    d_v = nc.declare_dram_parameter("v", [A], F32, isOutput=False)
    d_out = nc.declare_dram_parameter("out", [B_LOC, H], F32, isOutput=True)

    from contextlib import ExitStack

    with tile.TileContext(nc) as tc, ExitStack() as ctx:
        build_kernel_body(tc, d_query, d_keys, d_wq, d_wk, d_v, d_out, ctx)
    nc.compile()
    return nc


def build_kernel_body(tc, d_query, d_keys, d_wq, d_wk, d_v, d_out, ctx):
    nc = tc.nc

    consts = ctx.enter_context(tc.tile_pool(name="consts", bufs=1))
    keyp = ctx.enter_context(tc.tile_pool(name="keyp", bufs=4))
    keybf = ctx.enter_context(tc.tile_pool(name="keybf", bufs=4))
    keytp = ctx.enter_context(tc.tile_pool(name="keytp", bufs=4))
    tp = ctx.enter_context(tc.tile_pool(name="tp", bufs=3))
    smalls = ctx.enter_context(tc.tile_pool(name="smalls", bufs=4))
    pp_kt = ctx.enter_context(tc.tile_pool(name="pp_kt", bufs=3, space="PSUM"))
    pp_e = ctx.enter_context(tc.tile_pool(name="pp_e", bufs=2, space="PSUM"))
    pp_ctx = ctx.enter_context(tc.tile_pool(name="pp_ctx", bufs=3, space="PSUM"))

    # ---- constants ----
    # Wk in bf16, laid out [h' (part), hc, a]
    wk_bf = consts.tile([P, HC, A], BF16)
    nc.gpsimd.dma_start(
        out=wk_bf, in_=d_wq_rearr(d_wk)
    )  # SWDGE casts f32 -> bf16 in flight
    # Wq in bf16, same layout
    wq_sb = consts.tile([P, HC, A], BF16)
    nc.gpsimd.dma_start(out=wq_sb, in_=d_wq_rearr(d_wq))

    # v: load f32, DVE-cast into row 0 of a 16-row tile (single-producer
    # funnel so the xbar transpose carries only one wait), then xbar.
    v_f32 = consts.tile([1, A], F32)
    nc.gpsimd.dma_start(out=v_f32, in_=d_v[None, :])
    v16 = consts.tile([16, A], BF16)
    nc.vector.memset(v16, 0.0)
    nc.vector.tensor_copy(v16[0:1, :], v_f32)
    vT16 = consts.tile([P, AC, 16], BF16)
    nc.sync.dma_start(out=vT16, in_=v16, transpose=True)

    # query: same funnel pattern
    q_f32 = consts.tile([B_LOC, H], F32)
    nc.gpsimd.dma_start(out=q_f32, in_=d_query[:, :])
    q16 = consts.tile([16, H], BF16)
    nc.vector.memset(q16, 0.0)
    nc.vector.tensor_copy(q16[0:B_LOC, :], q_f32)
    qT16 = consts.tile([P, HC, 16], BF16)
    nc.sync.dma_start(out=qT16, in_=q16, transpose=True)

    # q_t = query @ Wq : psum (16, A), accumulate over hc
    ps_qt = pp_e.tile([16, A], F32, tag="pe")
    for hc in range(HC):
        nc.tensor.matmul(
            ps_qt,
            lhsT=qT16[:, hc, :],
            rhs=wq_sb[:, hc, :],
            start=(hc == 0),
            stop=(hc == HC - 1),
        )
    qt16 = consts.tile([16, A], BF16)
    nc.vector.memset(qt16, 0.0)
    nc.vector.tensor_copy(qt16[0:B_LOC, :], ps_qt[0:B_LOC, :])
    # xbar -> qtT16 (128, AC, 16); tanh bias per (ac, b) = qtT16[:, ac, b]
    qtT16 = consts.tile([P, AC, 16], BF16)
    nc.sync.dma_start(out=qtT16, in_=qt16, transpose=True)

    ones_bf = consts.tile([P, 1], BF16)
    nc.vector.memset(ones_bf, 1.0)

    # ---- main loop (2-stage pipelined emission: front i, compute i-1) ----
    iters = [(b, st) for b in range(B_LOC) for st in range(N_ST)]
    ctx_psums = {}
    front = {}
    front_loads = {}

    def stage_load(b, st):
        # load keys tile natural [s' (part), r, h] f32, then DVE-cast to bf16
        keys_nat = keyp.tile([P, SC, H], F32, tag="keys")
        nc.scalar.dma_start(
            out=keys_nat,
            in_=d_keys[b, st * ST : (st + 1) * ST, :].rearrange(
                "(p r) h -> p r h", p=P
            ),
        )
        keys_bf = keybf.tile([P, SC, H], BF16, tag="kbf")
        nc.vector.tensor_copy(keys_bf, keys_nat)
        return keys_bf

    def stage_xpose(b, st):
        keys_bf = front_loads[(b, st)]
        # transpose: keysT [h' (part), sc, hc, s']
        keysT = keytp.tile([P, SC, HC, P], BF16, tag="kT")
        for j in range(2):
            nc.sync.dma_start(
                out=keysT[:, 2 * j : 2 * j + 2, :, :],
                in_=keys_bf[:, 2 * j : 2 * j + 2, :],
                transpose=True,
            )
        return keys_bf, keysT

    def stage_compute(b, st):
        keys_bf, keysT = front.pop((b, st))

        first = st == 0
        last = st == N_ST - 1
        if first:
            ps_c0_new = pp_ctx.tile([1, 512], F32, tag="ctx")
            ps_c1_new = pp_ctx.tile([1, 512], F32, tag="ctx")
            ps_z_new = pp_ctx.tile([1, 1], F32, tag="ctx")
            ctx_psums[b] = (ps_c0_new, ps_c1_new, ps_z_new)
        ps_c0, ps_c1, _ = ctx_psums[b]

        # projection + tanh: T[a' (part), ac, s]
        T_sb = tp.tile([P, AC, ST], BF16, tag="T")
        for ac in range(AC):
            ps_kt = pp_kt.tile([P, ST], F32, tag="kt")
            for hc in range(HC):
                nc.tensor.matmul(
                    ps_kt,
                    lhsT=wk_bf[:, hc, ac * P : (ac + 1) * P],
                    rhs=keysT[:, :, hc, :],
                    start=(hc == 0),
                    stop=(hc == HC - 1),
                )
            nc.scalar.activation(
                T_sb[:, ac, :],
                ps_kt,
                mybir.ActivationFunctionType.Tanh,
                bias=qtT16[:, ac, b : b + 1],
            )

        # energy transposed: eT (128, SC) via regular matmuls (M=s chunk)
        ps_eT = pp_e.tile([P, SC], F32, tag="pe")
        for sc in range(SC):
            for ac in range(AC):
                nc.tensor.matmul(
                    ps_eT[:, sc : sc + 1],
                    lhsT=T_sb[:, ac, sc * P : (sc + 1) * P],
                    rhs=vT16[:, ac, 0:1],
                    start=(ac == 0),
                    stop=(ac == AC - 1),
                )

        # w^T = exp(eT) straight into SBUF, already s-on-partitions
        wT_sb = smalls.tile([P, SC], BF16, tag="wT")
        nc.scalar.activation(
            wT_sb,
            ps_eT,
            mybir.ActivationFunctionType.Exp,
        )

        # context accumulation: ctx (1, H) += w^T.T @ keys_bf
        # plus Z accumulation with a ones column (same bf16 weights as ctx)
        ps_z = ctx_psums[b][2]
        for sc in range(SC):
            nc.tensor.matmul(
                ps_c0,
                lhsT=wT_sb[:, sc : sc + 1],
                rhs=keys_bf[:, sc, 0:512],
                start=(first and sc == 0),
                stop=(last and sc == SC - 1),
            )
            nc.tensor.matmul(
                ps_c1,
                lhsT=wT_sb[:, sc : sc + 1],
                rhs=keys_bf[:, sc, 512:1024],
                start=(first and sc == 0),
                stop=(last and sc == SC - 1),
            )
            nc.tensor.matmul(
                ps_z,
                lhsT=wT_sb[:, sc : sc + 1],
                rhs=ones_bf[:, 0:1],
                start=(first and sc == 0),
                stop=(last and sc == SC - 1),
            )
        if last:
            finalize_batch(b, ps_c0, ps_c1, ctx_psums[b][2])

    def finalize_batch(b, ps_c0, ps_c1, ps_z):
        # finalize batch: out = ctx / Z
        rz = smalls.tile([1, 1], F32, tag="rz")
        nc.vector.reciprocal(rz, ps_z)
        out_sb = smalls.tile([1, H], F32, tag="out")
        nc.vector.tensor_scalar_mul(out_sb[0:1, 0:512], ps_c0, rz)
        nc.vector.tensor_scalar_mul(out_sb[0:1, 512:1024], ps_c1, rz)
        nc.gpsimd.dma_start(out=d_out[b : b + 1, :], in_=out_sb)

    n = len(iters)
    for i in range(n + 1):
        if i < n:
            front_loads[iters[i]] = stage_load(*iters[i])
            front[iters[i]] = stage_xpose(*iters[i])
            front_loads.pop(iters[i])
        if i >= 1:
            stage_compute(*iters[i - 1])


def d_wq_rearr(d_w):
    # (H, A) dram -> [h' (part), hc, a] view
    return d_w.rearrange("(hc p) a -> p hc a", p=P)
_CACHED_NC = None


def _get_nc():
    global _CACHED_NC
    if _CACHED_NC is None:
        _CACHED_NC = build_bass()
    return _CACHED_NC


def kernel(query, keys, Wq, Wk, v):
    query = np.ascontiguousarray(np.asarray(query, dtype=np.float32))
    keys = np.ascontiguousarray(np.asarray(keys, dtype=np.float32))
    Wq = np.ascontiguousarray(np.asarray(Wq, dtype=np.float32))
    Wk = np.ascontiguousarray(np.asarray(Wk, dtype=np.float32))
    v = np.ascontiguousarray(np.asarray(v, dtype=np.float32))

    nc = _get_nc()
    in_maps = []
    for c in range(N_CORES):
        sl = slice(c * B_LOC, (c + 1) * B_LOC)
        in_maps.append(
            {
                "query": query[sl],
                "keys": keys[sl],
                "Wq": Wq,
                "Wk": Wk,
                "v": v,
            }
        )
    last_err = None
    for attempt in range(3):
        try:
            res = run_bass_kernel_spmd(nc, in_maps, list(range(N_CORES)))
            out = np.concatenate(
                [np.asarray(res.results[c]["out"]) for c in range(N_CORES)], axis=0
            )
            break
        except Exception as e:  # transient device-unrecoverable states heal on retry
            last_err = e
            import time

            time.sleep(5)
    else:
        raise last_err
    return out.reshape(B, 1, H).astype(np.float32)


if __name__ == "__main__":
    rng = np.random.default_rng(0)
    q = rng.standard_normal((B, H), dtype=np.float32)
    k = rng.standard_normal((B, S, H), dtype=np.float32)
    wq = rng.standard_normal((H, A), dtype=np.float32) / np.sqrt(H)
    wk = rng.standard_normal((H, A), dtype=np.float32) / np.sqrt(H)
    vv = rng.standard_normal((A,), dtype=np.float32) / np.sqrt(A)
    o = kernel(query=q, keys=k, Wq=wq, Wk=wk, v=vv)
    print(o.shape, o.dtype)
